# revision 37
# baseline (speedup 1.0000x reference)
"""Trainium2 Bass kernel for nn_AttentionQKNorm (B=4, N=2048, C=1024, H=16, D=64).

Sharding (8 cores): core c -> batch b = c//2, head-group hg = c%2 (8 heads).
Tensor-parallel within a batch: each core computes qkv for its 8 heads,
per-head QK-LayerNorm, attention, and a partial projection
o_part @ w_proj[rows] -> [2048, 1024]. Host sums the two partials per batch.

v3 design (Act-engine-first): on HW the Act engine's exp stream
(256 x [128,1024] ~= 266us) is the binding resource, NOT the PE
(N=512 bf16 matmuls measure ~131ns, far below the naive 213ns model).
So the steady-state Act stream carries ONLY exp; everything else moves
to DVE/PE, and the ramp (k/v-side projection+LN, which gates block 1)
is split across DVE and Act:
  - qkv projection psums carry 8 extra "mean columns" (w @ per-head
    ones/64) so the per-token mean needs no DVE reduce.
  - LN chain per 128-token chunk: mu copy + sub -> qc, qc^2 (DVE mul for
    q-side, Act Square for the ramp-critical k-side), grouped reduce,
    batched rstd (one Ln+Exp per 4-chunk group on Act), apply mul.
  - transpose evacs: Act Copy-with-scale-AP (gamma) for k-side,
    DVE tensor_scalar_mul for q-side.
  - V psum evac: ONE strided DVE copy per chunk into vx_all
    (data cols 0:64 of each 65-col block; ones col via strided memset);
    ramp-critical odd chunks go through Act instead.
  - divide: DVE copy PV psum -> ocp (frees pv bank), DVE recip,
    PE ones-matmul broadcast, DVE mul. No Act copies.
  - blocks are qb-major so the projection tail spreads over blocks 5+.
  - per-step emission order: S MMs, chunk MMs (PE fillers), exp+PV,
    then DVE/Act filler tails -- keeps PE fed during exp and keeps the
    Act FIFO free of ops that could head-block the next exp.

All matmul operands bf16 (host pre-converts); f32 psum accumulation.
"""

import numpy as np

H = 16
D = 64
B = 4
SEQ = 2048
C = 1024
NCORES = 8
NHP = 4  # head-pairs per core (8 heads)
EPS = 1e-6
SCALE = D**-0.5

_CACHE = {}

# timing-ablation switch (used by ablate.py only):
#   None: full kernel; "pipe": S/exp/PV only; "div": +divide; "proj": +proj
_ABLATE = None
# divide-chain ablation: "full" | "ocp" | "recip" | "psr" (cumulative stages)
_DIVMODE = "full"
_RECIP_EXACT = False


def _build_nc(reps=1, with_bias=False):
    ab = _ABLATE
    from contextlib import ExitStack

    import concourse.bacc as bacc
    import concourse.tile as tile
    import concourse.mybir as mybir

    dt = mybir.dt
    F32, F32R, BF = dt.float32, dt.float32r, dt.bfloat16
    AF = mybir.ActivationFunctionType
    ALU = mybir.AluOpType
    AX = mybir.AxisListType

    # Ln and Exp both live in act-table set "natural_log_exp_and_others"
    # (which also has Square/Copy/Identity), but the table-load inserter
    # picks the FIRST set containing each func, causing a ~2.7us table
    # reload around every LayerNorm rstd. Hide Ln/Exp from the earlier
    # sets so both resolve to the shared set -> one load total.
    if not getattr(bacc, "_qknorm_act_tables_patched", False):
        _orig_get_tables = bacc.get_activation_tables

        def _patched_get_tables(arch):
            tabs = {k: set(v) for k, v in _orig_get_tables(arch).items()}
            af = mybir.ActivationFunctionType
            both = "natural_log_exp_and_others"
            if both in tabs and af.Exp in tabs[both] and af.Ln in tabs[both]:
                for name, funcs in tabs.items():
                    if name != both:
                        funcs.discard(af.Ln)
                        funcs.discard(af.Exp)
            return tabs

        bacc.get_activation_tables = _patched_get_tables
        bacc._qknorm_act_tables_patched = True

    nc = bacc.Bacc("TRN2", target_bir_lowering=False, debug=False,
                   num_devices=NCORES)

    def din(name, shape, dtype=BF):
        return nc.dram_tensor(name, shape, dtype, kind="ExternalInput").ap()

    xT = din("xT", [C, SEQ])
    wq = din("wq", [C, 512])
    wk = din("wk", [C, 512])
    wv = din("wv", [C, 512])
    wp = din("wp", [512, C])
    bqk = din("bqk", [2, 512])          # rows: bq, bk (this half)
    onescol = din("onescol", [1, 128])  # bias-prefill lhsT
    ident = din("ident", [128, 128])    # transpose rhs
    gqc = din("gqc", [128, 1], F32)     # g_q per d-row of a head pair
    gkc = din("gkc", [128, 1], F32)
    ones1 = din("ones1", [1, 128], F32R)
    epsc = din("epsc", [128, 1], F32)
    bp = din("bp", [128, 8], F32)       # b_proj + bv@wp, [128, m]
    outT = nc.dram_tensor("outT", [C, SEQ], F32, kind="ExternalOutput").ap()

    with tile.TileContext(nc) as tc, ExitStack() as ctx, \
            nc.allow_low_precision("bf16 matmul operands by design"):
        ep = ctx.enter_context

        const_p = ep(tc.tile_pool(name="const", bufs=1))
        xt_p = ep(tc.tile_pool(name="xt", bufs=1))      # 32KB/p
        w_p = ep(tc.tile_pool(name="w", bufs=1))        # ~24.5KB/p
        qkT_p = ep(tc.tile_pool(name="qkT", bufs=1))    # 32KB/p
        vx_p = ep(tc.tile_pool(name="vx", bufs=1))      # 16.3KB/p
        qc_p = ep(tc.tile_pool(name="qc", bufs=7))      # 7 x 2KB/p
        st_p = ep(tc.tile_pool(name="st", bufs=2))      # small stats tiles
        tm_p = ep(tc.tile_pool(name="tm", bufs=5))      # 5 x 1KB/p
        pg_p = ep(tc.tile_pool(name="pg", bufs=6))      # 6 x 2KB/p
        oT_p = ep(tc.tile_pool(name="oT", bufs=1))      # 16KB/p
        scr_p = ep(tc.tile_pool(name="scr", bufs=4))
        wpm_p = ep(tc.tile_pool(name="wpm", bufs=1))    # 8KB/p
        # ps_big ([128,1024] x2 = 4 banks) is EXCLUSIVELY the S psums so the
        # S->exp double-buffering never waits on filler work; chunks,
        # transposes, recip-broadcasts and proj all rotate through ps_msc
        # ([128,512] x2 = 2 banks).
        ps_big = ep(tc.tile_pool(name="psb", bufs=2, space="PSUM"))  # 4 banks
        ps_pv = ep(tc.tile_pool(name="pspv", bufs=2, space="PSUM"))  # 2 banks
        ps_msc = ep(tc.tile_pool(name="psmsc", bufs=2, space="PSUM"))  # 2 banks

        for rep in range(reps):
            # ---- constants (DMAs deferred until after the k-path loads) ----
            _const_dmas = []

            def cst(shape, dtype, tag, src):
                t = const_p.tile(shape, dtype, tag=tag, name=f"r{rep}{tag}")
                _const_dmas.append((t, src))
                return t

            def emit_consts():
                for t, src in _const_dmas:
                    nc.sync.dma_start(t[:], src)
                _const_dmas.clear()

            ones_sb = cst([1, 128], BF, "onescol", onescol)
            id_sb = cst([128, 128], BF, "ident", ident)
            bq_sb = cst([1, 512], BF, "bqr", bqk[0:1, :])
            bk_sb = cst([1, 512], BF, "bkr", bqk[1:2, :])
            gq_sb = cst([128, 1], F32, "gqc", gqc)
            gk_sb = cst([128, 1], F32, "gkc", gkc)
            ones1_sb = cst([1, 128], F32R, "ones1", ones1)
            eps_sb = cst([128, 1], F32, "epsc", epsc)
            bp_sb = cst([128, 8], F32, "bp", bp)

            # DMA order matters: k-path inputs (wk + x band0) first so the
            # prologue's first matmuls aren't queued behind wq/wv/late bands
            wsb = {}
            _wdrams = {"k": wk, "q": wq, "v": wv}

            def load_w(wname):
                wt = w_p.tile([128, 8 * 512], BF, tag=f"w{wname}",
                              name=f"r{rep}w{wname}")
                nc.sync.dma_start(
                    wt[:].rearrange("p (kc c) -> p kc c", kc=8),
                    _wdrams[wname].rearrange("(kc p) c -> p kc c", p=128))
                wsb[wname] = [wt[:, kc * 512:(kc + 1) * 512]
                              for kc in range(8)]

            xt_all = xt_p.tile([128, 8 * SEQ], BF, tag="xt",
                               name=f"r{rep}xt")
            xt_sb = [xt_all[:, kc * SEQ:(kc + 1) * SEQ] for kc in range(8)]

            def load_band(lo, hi):
                nc.sync.dma_start(
                    xt_all[:].rearrange("p (kc t) -> p kc t", kc=8)
                    [:, :, lo:hi],
                    xT.rearrange("(kc p) t -> p kc t", p=128)[:, :, lo:hi])

            load_w("k")
            load_band(0, 128)       # unblocks k0 asap
            emit_consts()
            load_band(128, 512)
            load_w("q")
            load_band(512, 1024)
            load_w("v")
            load_band(1024, 1536)
            load_band(1536, 2048)

            # persistent destination tiles
            qnT = {}
            for hp in range(NHP):
                for wname in ("q", "k"):
                    qnT[hp, wname] = qkT_p.tile(
                        [128, SEQ], BF, tag=f"{wname}T{hp}",
                        name=f"r{rep}{wname}T{hp}")
            # vx_all: per hp 16 kc-chunks of (64 v-cols + ones col) per head
            vx_all = vx_p.tile([128, NHP * 16 * 130], BF, tag="vx",
                               name=f"r{rep}vx")
            # ones columns only (col 64 of every 65-block)
            nc.vector.memset(
                vx_all[:].rearrange("p (b s) -> p b s", s=65)[:, :, 64:65],
                1.0)

            def vx_sl(hp, kc, h):
                base = hp * 2080 + kc * 130 + 65 * h
                return vx_all[:, base:base + 65]

            oTs = {}
            for hp in range(NHP):
                oTs[hp] = oT_p.tile([128, SEQ], BF, tag=f"oT{hp}",
                                    name=f"r{rep}oT{hp}")

            # ---- qkv chunk machinery ----
            ps_chunks = {}     # (w, t16) -> psum tile
            qc_tiles = {}      # (w, t16) -> centered values, f32
            sq_tiles = {}      # (w, g4) -> [128, 32] grouped sum-of-squares
            mu_tiles = {}

            def chunk_mm(wname, t16):
                """Projection matmuls for tok-chunk t16 (PE part)."""
                ts = slice(t16 * 128, (t16 + 1) * 128)
                ps = ps_msc.tile([128, 512], F32, tag="msc",
                                 name=f"r{rep}ps{wname}{t16}")
                if with_bias and wname != "v":
                    b_sb = bq_sb if wname == "q" else bk_sb
                    nc.tensor.matmul(ps[:], lhsT=ones_sb[:],
                                     rhs=b_sb[:], start=True, stop=False)
                for kc in range(8):
                    nc.tensor.matmul(
                        ps[:],
                        lhsT=xt_sb[kc][:, ts],
                        rhs=wsb[wname][kc][:],
                        start=(kc == 0 and not (with_bias and wname != "v")),
                        stop=(kc == 7),
                    )
                ps_chunks[wname, t16] = ps

            def qk_sub(wname, t16):
                """Per-head sums + centered values tdif = 64*q - sums (DVE).
                The 64 scaling folds into gamma (g/64) at transpose evac."""
                ps = ps_chunks[wname, t16]
                qb3 = ps[:].rearrange("p (g d) -> p g d", d=D)
                mus = st_p.tile([128, 8], F32, tag="mus", bufs=4,
                                name=f"r{rep}mus{wname}{t16}")
                nc.vector.tensor_reduce(mus[:], qb3, AX.X, ALU.add)
                qc = qc_p.tile([128, 512], F32, tag="qc",
                               name=f"r{rep}qc{wname}{t16}")
                mus_v = mus[:].unsqueeze(-1).broadcast_to([128, 8, D])
                nc.vector.scalar_tensor_tensor(
                    qc[:].rearrange("p (g d) -> p g d", d=D),
                    qb3, float(D), mus_v,
                    op0=ALU.mult, op1=ALU.subtract)
                qc_tiles[wname, t16] = qc
                mu_tiles[wname, t16] = mus

            def qk_sq(wname, t16, on_act):
                """qc^2 -> q2; grouped reduce into the group's sq tile."""
                qc = qc_tiles[wname, t16]
                g4, j = t16 // 4, t16 % 4
                if (wname, g4) not in sq_tiles:
                    sq_tiles[wname, g4] = st_p.tile(
                        [128, 32], F32, tag="sq", bufs=3,
                        name=f"r{rep}sq{wname}{g4}")
                q2 = scr_p.tile([128, 512], F32, tag="q2", bufs=3,
                                name=f"r{rep}q2{wname}{t16}")
                if on_act:
                    nc.scalar.square(q2[:], qc[:])
                else:
                    nc.vector.tensor_mul(q2[:], qc[:], qc[:])
                return q2

            def qk_red(wname, t16, q2):
                g4, j = t16 // 4, t16 % 4
                nc.vector.tensor_reduce(
                    sq_tiles[wname, g4][:, 8 * j:8 * j + 8],
                    q2[:].rearrange("p (g d) -> p g d", d=D),
                    AX.X, ALU.add)

            def qk_stats(wname, t16, on_act=False):
                qk_sub(wname, t16)
                qk_red(wname, t16, qk_sq(wname, t16, on_act))

            def tr_group(wname, g4, evac_act=False):
                """Finalize tok-chunks 4*g4..4*g4+3 of q|k: batched rstd,
                apply (DVE), PE-transpose into qnT, evac applies gamma."""
                g_sb = gq_sb if wname == "q" else gk_sb
                sq = sq_tiles.pop((wname, g4))
                lnv = st_p.tile([128, 32], F32, tag="lnv", bufs=2,
                                name=f"r{rep}lnv{wname}{g4}")
                nc.scalar.activation(lnv[:], sq[:], AF.Ln,
                                     bias=eps_sb[:, 0:1],
                                     scale=1.0 / (D * D * D))
                rstd = st_p.tile([128, 32], F32, tag="rstd", bufs=2,
                                 name=f"r{rep}rstd{wname}{g4}")
                nc.scalar.activation(rstd[:], lnv[:], AF.Exp, scale=-0.5)
                tms = []
                for j in range(4):
                    t16 = 4 * g4 + j
                    qc = qc_tiles.pop((wname, t16))
                    mu_tiles.pop((wname, t16), None)
                    rstd_v = (rstd[:, 8 * j:8 * j + 8]
                              .unsqueeze(-1).broadcast_to([128, 8, D]))
                    tm = tm_p.tile([128, 512], BF, tag="tm",
                                   name=f"r{rep}tm{wname}{t16}")
                    nc.vector.tensor_mul(
                        tm[:].rearrange("p (g d) -> p g d", d=D),
                        qc[:].rearrange("p (g d) -> p g d", d=D), rstd_v)
                    tms.append(tm)
                for hp in range(NHP):
                    pstf = ps_msc.tile([128, 512], F32, tag="msc",
                                       name=f"r{rep}tr{wname}{g4}{hp}")
                    pst = pstf[:].bitcast(BF)[:, 0:512]
                    for j in range(4):
                        nc.tensor.transpose(
                            pst[:, j * 128:(j + 1) * 128],
                            tms[j][:, hp * 128:(hp + 1) * 128], id_sb[:])
                    dst = qnT[hp, wname][:, g4 * 512:(g4 + 1) * 512]
                    if evac_act:
                        nc.scalar.activation(dst, pst[:], AF.Copy,
                                             scale=g_sb[:, 0:1])
                    else:
                        nc.vector.tensor_scalar_mul(dst, pst[:],
                                                    g_sb[:, 0:1])

            def v_copy(t16, on_act=False):
                """V psum -> vx_all slices, one strided copy (all hp)."""
                ps = ps_chunks["v", t16]
                # dst: [128, hp, 2, 64] = data cols of the two 65-blocks
                dst = (vx_all[:]
                       .rearrange("p (hp kc s) -> p hp kc s", hp=NHP, kc=16)
                       [:, :, t16, :]
                       .rearrange("p hp (two s) -> p hp two s", s=65)
                       [:, :, :, 0:64])
                src = ps[:].rearrange(
                    "p (hp two s) -> p hp two s", hp=NHP, s=64)
                if on_act:
                    nc.scalar.copy(dst, src)
                else:
                    nc.vector.tensor_copy(dst, src)

            # ---- attention pipeline ----
            pvs_all = {}
            ps_tiles = {}

            def emit_s(hp, qb_i, g, h):
                qT, kT = qnT[hp, "q"], qnT[hp, "k"]
                qs = slice(qb_i * 512, (qb_i + 1) * 512)
                ps_s = ps_big.tile([128, 1024], F32, tag="big",
                                   name=f"r{rep}pss{hp}{qb_i}{g}{h}")
                for j in range(2):
                    kc = 2 * g + j
                    nc.tensor.matmul(
                        ps_s[:, j * 512:(j + 1) * 512],
                        lhsT=kT[slice(64 * h, 64 * h + 64),
                                kc * 128:(kc + 1) * 128],
                        rhs=qT[slice(64 * h, 64 * h + 64), qs],
                        start=True, stop=True,
                    )
                ps_tiles[hp, qb_i, g, h] = ps_s

            def emit_exp(hp, qb_i, g, h):
                ps_s = ps_tiles.pop((hp, qb_i, g, h))
                pg = pg_p.tile([128, 1024], BF, tag="pg",
                               name=f"r{rep}pg{hp}{qb_i}{g}{h}")
                nc.scalar.activation(pg[:], ps_s[:], AF.Exp, scale=SCALE)
                return pg

            def emit_pv(pg, hp, qb_i, g, h):
                pvs = pvs_all[hp, qb_i]
                for j in range(2):
                    kc = 2 * g + j
                    nc.tensor.matmul(
                        pvs[h][0:65, :],
                        lhsT=vx_sl(hp, kc, h),
                        rhs=pg[:, j * 512:(j + 1) * 512],
                        start=(kc == 0), stop=(kc == 15),
                    )

            if ab is not None:
                sink = scr_p.tile([1, 16], F32, tag="sink", bufs=1,
                                  name=f"r{rep}sink")
                nc.vector.memset(sink[:], 0.0)

            def _sink(ap):
                nc.vector.tensor_add(sink[:], sink[:], ap)

            def emit_divide_a(hp, qb_i):
                """Stage A at block end: evacuate PV psums on Act (frees the
                pv banks for the next block), batched fast-reciprocal of both
                denominator rows (DVE custom op; HW InstReciprocal costs
                ~2.4us), bf16 cast for the broadcast matmul."""
                pvs = pvs_all.pop((hp, qb_i))
                ocps = []
                rhbs = []
                for h in range(2):
                    ocp = scr_p.tile([128, 512], F32, tag="ocp", bufs=3,
                                     name=f"r{rep}ocp{hp}{qb_i}{h}")
                    nc.scalar.copy(ocp[0:64, :], pvs[h][0:64, :])
                    rhb = scr_p.tile([1, 512], BF, tag="rhb", bufs=3,
                                     name=f"r{rep}rhb{hp}{qb_i}{h}")
                    if _RECIP_EXACT:
                        rcp = scr_p.tile([1, 512], F32, tag="rcp", bufs=3,
                                         name=f"r{rep}rcp{hp}{qb_i}{h}")
                        nc.vector.reciprocal(rcp[:].bitcast(F32R),
                                             pvs[h][64:65, :])
                        nc.vector.tensor_copy(rhb[:], rcp[:])
                    else:
                        # 1/x = exp(-ln(x)) on Act: two tiny [1,512] ops from
                        # the already-loaded table set (HW InstReciprocal on
                        # DVE costs ~2.4us; the custom-op approx is broken on
                        # HW). The Exp writes the bf16 broadcast operand
                        # directly.
                        lnd = scr_p.tile([1, 512], F32, tag="lnd", bufs=3,
                                         name=f"r{rep}lnd{hp}{qb_i}{h}")
                        nc.scalar.activation(lnd[:], pvs[h][64:65, :],
                                             AF.Ln)
                        nc.scalar.activation(rhb[:], lnd[:], AF.Exp,
                                             scale=-1.0)
                    ocps.append(ocp)
                    rhbs.append(rhb)
                return (hp, qb_i, ocps, rhbs)

            def emit_divide_b(hp, qb_i, ocps, rhbs):
                """Stage B one iteration later (deps long fired): bf16
                ones-matmul broadcast of 1/denom, divide-mul into oT."""
                oT = oTs[hp]
                qs = slice(qb_i * 512, (qb_i + 1) * 512)
                for h in range(2):
                    psr = ps_msc.tile([128, 512], F32, tag="msc",
                                      name=f"r{rep}psrb{hp}{qb_i}{h}")
                    nc.tensor.matmul(
                        psr[0:64, :],
                        lhsT=ones_sb[0:1, 0:64], rhs=rhbs[h][:],
                        start=True, stop=True,
                    )
                    nc.vector.tensor_mul(
                        oT[64 * h:64 * h + 64, qs],
                        ocps[h][0:64, :], psr[0:64, :])

            # ---- projection ----
            wpm_all = wpm_p.tile([128, 8 * 4 * 128], BF, tag="wpm",
                                 name=f"r{rep}wpmall")

            def load_wpm():
                nc.sync.dma_start(
                    wpm_all[:].rearrange("p (m hp c) -> p m hp c",
                                         m=8, hp=4),
                    wp.rearrange("(hp p) (m c) -> p m hp c", p=128, c=128))

            def proj_group(m, n):
                ps = ps_msc.tile([128, 512], F32, tag="msc",
                                 name=f"r{rep}pspr{m}{n}")
                for hp in range(NHP):
                    nc.tensor.matmul(
                        ps[:],
                        lhsT=wpm_all[:, (m * 4 + hp) * 128:
                                     (m * 4 + hp + 1) * 128],
                        rhs=oTs[hp][:, n * 512:(n + 1) * 512],
                        start=(hp == 0), stop=(hp == NHP - 1),
                    )
                so = scr_p.tile([128, 512], F32, tag="so", bufs=4,
                                name=f"r{rep}so{m}{n}")
                nc.scalar.activation(so[:], ps[:], AF.Identity,
                                     bias=bp_sb[:, m:m + 1])
                nc.sync.dma_start(
                    outT[m * 128:(m + 1) * 128,
                         n * 512:(n + 1) * 512], so[:])

            # ---- prologue: k0-5, tr_k(0), q0-3, tr_q(0), v0-1 ----
            if ab is None:
                for t16 in range(4):
                    chunk_mm("k", t16)
                    qk_stats("k", t16)
                tr_group("k", 0, evac_act=True)
                for t16 in range(4):
                    chunk_mm("q", t16)
                    qk_stats("q", t16)
                tr_group("q", 0, evac_act=True)
                for t16 in (4, 5):
                    chunk_mm("k", t16)
                    qk_stats("k", t16)
                chunk_mm("v", 0)
                v_copy(0, on_act=True)
                chunk_mm("v", 1)
                v_copy(1, on_act=True)
            else:
                # ablation: fake qnT/vx contents, skip all LN machinery
                for hp in range(NHP):
                    for wname in ("q", "k"):
                        nc.vector.memset(qnT[hp, wname][:], 0.05)
                nc.vector.memset(vx_all[:], 0.01)
                load_wpm()

            # ---- 256 attention steps, qb-major blocks ----
            steps = [(hp, qb_i, g, h)
                     for qb_i in range(4) for hp in range(NHP)
                     for g in range(8) for h in range(2)]
            fill_pre = {}    # PE-heavy fillers: run between S and exp
            fill_post = {}   # DVE/Act fillers: run after exp+PV

            def pre(i, fn):
                fill_pre.setdefault(i, []).append(fn)

            def post(i, fn):
                fill_post.setdefault(i, []).append(fn)

            # k chunks 6..15 stats pipelined: mm@s, sub@s, sq(Act)@s+1,
            # red@s+2.  tr_k(gi) needs chunks 4gi..4gi+3 reduced by step 4gi.
            q2_tiles = {}
            for idx, t16 in enumerate(range(6, 16) if ab is None else ()):
                s = idx  # steps 0..9
                pre(s, lambda t=t16: chunk_mm("k", t))
                post(s, lambda t=t16: qk_sub("k", t))
                post(s + 1, lambda t=t16: q2_tiles.__setitem__(
                    t, qk_sq("k", t, False)))
                post(s + 2, lambda t=t16: qk_red(
                    "k", t, q2_tiles.pop(t)))
            if ab is None:
                post(3, lambda: tr_group("k", 1, evac_act=True))
                post(7, lambda: tr_group("k", 2, evac_act=True))
                post(11, lambda: tr_group("k", 3, evac_act=True))

            # v chunks 2..15: needed by PV at iteration t+2
            for t16 in (range(2, 16) if ab is None else ()):
                s = t16 - 2  # steps 0..13
                pre(s, lambda t=t16: chunk_mm("v", t))
                post(s + 1, lambda t=t16: v_copy(t, on_act=True))

            # q chunks 4..15, pipelined like k; groups due at 64*g4
            for g4 in (range(1, 4) if ab is None else ()):
                base = 16 * g4
                for j in range(4):
                    t16 = 4 * g4 + j
                    pre(base + 3 * j, lambda t=t16: chunk_mm("q", t))
                    post(base + 3 * j, lambda t=t16: qk_sub("q", t))
                    post(base + 3 * j + 1, lambda t=t16: q2_tiles.__setitem__(
                        ("q", t), qk_sq("q", t, False)))
                    post(base + 3 * j + 2, lambda t=t16: qk_red(
                        "q", t, q2_tiles.pop(("q", t))))
                post(base + 12, lambda g=g4: tr_group("q", g, evac_act=True))

            if ab is None:
                pre(40, load_wpm)

            # Pipeline: S@i, exp@i-1, PV@i-2 -- by the time PV(i-2) and
            # S(i) reach the PE FIFO their sems have already fired, so the
            # PE never stalls mid-stream and the exp stream runs at the Act
            # engine's native rate.
            proj_pending = []
            pend_exp = None          # step awaiting exp
            pend_pv = None           # (pg, step) awaiting PV
            pend_div = []            # divide stage-B args, run next iteration

            def do_pv(pg, st):
                emit_pv(pg, *st)
                if st[2:] == (7, 1) and ab != "pipe":
                    php, pqb = st[:2]
                    pend_div.append(emit_divide_a(php, pqb))
                    if php == NHP - 1 and pqb < 3 and ab in (None, "proj"):
                        for m in range(8):
                            proj_pending.append(
                                lambda m=m, n=pqb: proj_group(m, n))

            for i, st in enumerate(steps):
                hp, qb_i, g, h = st
                if (g, h) == (0, 0):
                    pvs_all[hp, qb_i] = [
                        ps_pv.tile([128, 512], F32, tag="pv",
                                   name=f"r{rep}pv{hp}{qb_i}{_h}")
                        for _h in range(2)]
                emit_s(*st)
                if pend_div:
                    emit_divide_b(*pend_div.pop(0))
                for t in fill_pre.get(i, ()):
                    t()
                if pend_pv is not None:
                    do_pv(*pend_pv)
                    pend_pv = None
                if pend_exp is not None:
                    pend_pv = (emit_exp(*pend_exp), pend_exp)
                pend_exp = st
                for t in fill_post.get(i, ()):
                    t()
                if proj_pending and i % 2 == 0:
                    proj_pending.pop(0)()
            # drain
            last_pg = emit_exp(*pend_exp)
            do_pv(*pend_pv)
            do_pv(last_pg, pend_exp)
            for d in pend_div:
                emit_divide_b(*d)
            for t in proj_pending:
                t()
            if ab in (None, "proj"):
                for m in range(8):
                    proj_group(m, 3)
            elif ab == "pipe":
                for h in range(2):
                    sod = scr_p.tile([128, 512], F32, tag="so", bufs=4,
                                     name=f"r{rep}abl{h}")
                    nc.vector.tensor_copy(sod[:], pvs_all[3, 3][h][:])
                    nc.sync.dma_start(outT[128 * h:128 * (h + 1), 0:512],
                                      sod[:])
            elif ab == "div":
                if _DIVMODE == "full":
                    for hp in range(NHP):
                        sod = scr_p.tile([128, 512], F32, tag="so", bufs=4,
                                         name=f"r{rep}abl{hp}")
                        nc.vector.tensor_copy(sod[:], oTs[hp][:, 0:512])
                        nc.sync.dma_start(
                            outT[128 * hp:128 * (hp + 1), 0:512], sod[:])
                else:
                    nc.sync.dma_start(outT[0:1, 0:16], sink[:])

    nc.compile()
    return nc


def make_in_maps(x, w_qkv, b_qkv, g_q, g_k, w_proj, b_proj):
    """Host-side sharding: per-core input dict (weights pre-cast to bf16)."""
    import ml_dtypes
    f32 = np.float32
    bf16 = ml_dtypes.bfloat16
    x = np.ascontiguousarray(x, dtype=f32)
    w_qkv = np.asarray(w_qkv, dtype=f32)
    b_qkv = np.asarray(b_qkv, dtype=f32)
    g_q = np.asarray(g_q, dtype=f32)
    g_k = np.asarray(g_k, dtype=f32)
    w_proj = np.asarray(w_proj, dtype=f32)
    b_proj = np.asarray(b_proj, dtype=f32)

    ident = np.eye(128, dtype=f32)
    onescol = np.ones((1, 128), f32)
    ones1 = np.ones((1, 128), f32)
    # g/64 per d-row of a head pair (folds the tdif = 64*(q-mu) scaling)
    gqc = np.concatenate([g_q, g_q]).reshape(128, 1) * (1.0 / 64.0)
    gkc = np.concatenate([g_k, g_k]).reshape(128, 1) * (1.0 / 64.0)

    in_maps = []
    for c in range(NCORES):
        b = c // 2
        hg = c % 2
        cs = slice(hg * 512, (hg + 1) * 512)
        bv = b_qkv[2 * C:][cs]
        wp_half = w_proj[hg * 512:(hg + 1) * 512, :]
        bp_eff = bv @ wp_half + (b_proj if hg == 0 else 0.0)
        bqk_rows = np.stack([b_qkv[cs], b_qkv[C:][cs]])
        in_maps.append({
            "xT": np.ascontiguousarray(x[b].T).astype(bf16),
            "wq": np.ascontiguousarray(w_qkv[:, cs]).astype(bf16),
            "wk": np.ascontiguousarray(w_qkv[:, C:][:, cs]).astype(bf16),
            "wv": np.ascontiguousarray(w_qkv[:, 2 * C:][:, cs]).astype(bf16),
            "wp": np.ascontiguousarray(wp_half).astype(bf16),
            "bqk": bqk_rows.astype(bf16),
            "onescol": onescol.astype(bf16),
            "ident": ident.astype(bf16),
            "gqc": gqc,
            "gkc": gkc,
            "ones1": ones1,
            "epsc": np.full((128, 1), EPS, f32),
            "bp": np.ascontiguousarray(bp_eff.reshape(8, 128).T.astype(f32)),
        })
    return in_maps


def unshard(partials):
    """partials: list of 8 outT arrays [C, SEQ] -> full [B, SEQ, C]."""
    out = np.empty((B, SEQ, C), np.float32)
    for b in range(B):
        out[b] = (partials[2 * b] + partials[2 * b + 1]).T
    return out


def kernel(x, w_qkv, b_qkv, g_q, g_k, w_proj, b_proj):
    from concourse.bass_utils import run_bass_kernel_spmd

    # q/k biases feed the pre-LN values; emit the bias-prefill matmuls only
    # if they are actually nonzero (the spec fills b_qkv with zeros)
    wb = bool(np.any(np.asarray(b_qkv)[:2 * C]))
    key = ("nc", wb)
    if key not in _CACHE:
        _CACHE[key] = _build_nc(with_bias=wb)
    nc = _CACHE[key]
    in_maps = make_in_maps(x, w_qkv, b_qkv, g_q, g_k, w_proj, b_proj)
    res = run_bass_kernel_spmd(nc, in_maps, list(range(NCORES)))
    return unshard([res.results[c]["outT"] for c in range(NCORES)])


# revision 38
# speedup vs baseline: 1.1541x; 1.1541x over previous
"""Trainium2 Bass kernel for nn_AttentionQKNorm (B=4, N=2048, C=1024, H=16, D=64).

Sharding (8 cores): core c -> batch b = c//2, head-group hg = c%2 (8 heads).
Tensor-parallel within a batch: each core computes qkv for its 8 heads,
per-head QK-LayerNorm, attention, and a partial projection
o_part @ w_proj[rows] -> [2048, 1024]. Host sums the two partials per batch.

v3 design (Act-engine-first): on HW the Act engine's exp stream
(256 x [128,1024] ~= 266us) is the binding resource, NOT the PE
(N=512 bf16 matmuls measure ~131ns, far below the naive 213ns model).
So the steady-state Act stream carries ONLY exp; everything else moves
to DVE/PE, and the ramp (k/v-side projection+LN, which gates block 1)
is split across DVE and Act:
  - qkv projection psums carry 8 extra "mean columns" (w @ per-head
    ones/64) so the per-token mean needs no DVE reduce.
  - LN chain per 128-token chunk: mu copy + sub -> qc, qc^2 (DVE mul for
    q-side, Act Square for the ramp-critical k-side), grouped reduce,
    batched rstd (one Ln+Exp per 4-chunk group on Act), apply mul.
  - transpose evacs: Act Copy-with-scale-AP (gamma) for k-side,
    DVE tensor_scalar_mul for q-side.
  - V psum evac: ONE strided DVE copy per chunk into vx_all
    (data cols 0:64 of each 65-col block; ones col via strided memset);
    ramp-critical odd chunks go through Act instead.
  - divide: DVE copy PV psum -> ocp (frees pv bank), DVE recip,
    PE ones-matmul broadcast, DVE mul. No Act copies.
  - blocks are qb-major so the projection tail spreads over blocks 5+.
  - per-step emission order: S MMs, chunk MMs (PE fillers), exp+PV,
    then DVE/Act filler tails -- keeps PE fed during exp and keeps the
    Act FIFO free of ops that could head-block the next exp.

All matmul operands bf16 (host pre-converts); f32 psum accumulation.
"""

import numpy as np

H = 16
D = 64
B = 4
SEQ = 2048
C = 1024
NCORES = 8
NHP = 4  # head-pairs per core (8 heads)
EPS = 1e-6
SCALE = D**-0.5

_CACHE = {}

# timing-ablation switch (used by ablate.py only):
#   None: full kernel; "pipe": S/exp/PV only; "div": +divide; "proj": +proj
_ABLATE = None
# divide-chain ablation: "full" | "ocp" | "recip" | "psr" (cumulative stages)
_DIVMODE = "full"
_RECIP_EXACT = False


def _build_nc(reps=1, with_bias=False):
    ab = _ABLATE
    from contextlib import ExitStack

    import concourse.bacc as bacc
    import concourse.tile as tile
    import concourse.mybir as mybir

    dt = mybir.dt
    F32, F32R, BF = dt.float32, dt.float32r, dt.bfloat16
    AF = mybir.ActivationFunctionType
    ALU = mybir.AluOpType
    AX = mybir.AxisListType

    # Ln and Exp both live in act-table set "natural_log_exp_and_others"
    # (which also has Square/Copy/Identity), but the table-load inserter
    # picks the FIRST set containing each func, causing a ~2.7us table
    # reload around every LayerNorm rstd. Hide Ln/Exp from the earlier
    # sets so both resolve to the shared set -> one load total.
    if not getattr(bacc, "_qknorm_act_tables_patched", False):
        _orig_get_tables = bacc.get_activation_tables

        def _patched_get_tables(arch):
            tabs = {k: set(v) for k, v in _orig_get_tables(arch).items()}
            af = mybir.ActivationFunctionType
            both = "natural_log_exp_and_others"
            if both in tabs and af.Exp in tabs[both] and af.Ln in tabs[both]:
                for name, funcs in tabs.items():
                    if name != both:
                        funcs.discard(af.Ln)
                        funcs.discard(af.Exp)
            return tabs

        bacc.get_activation_tables = _patched_get_tables
        bacc._qknorm_act_tables_patched = True

    nc = bacc.Bacc("TRN2", target_bir_lowering=False, debug=False,
                   num_devices=NCORES)

    def din(name, shape, dtype=BF):
        return nc.dram_tensor(name, shape, dtype, kind="ExternalInput").ap()

    xT = din("xT", [C, SEQ])
    wq = din("wq", [C, 512])
    wk = din("wk", [C, 512])
    wv = din("wv", [C, 512])
    wp = din("wp", [512, C])
    bqk = din("bqk", [2, 512])          # rows: bq, bk (this half)
    onescol = din("onescol", [1, 128])  # bias-prefill lhsT
    ident = din("ident", [128, 128])    # transpose rhs
    gqc = din("gqc", [128, 1], F32)     # g_q per d-row of a head pair
    gkc = din("gkc", [128, 1], F32)
    ones1 = din("ones1", [1, 128], F32R)
    epsc = din("epsc", [128, 1], F32)
    bp = din("bp", [128, 8], F32)       # b_proj + bv@wp, [128, m]
    outT = nc.dram_tensor("outT", [C, SEQ], F32, kind="ExternalOutput").ap()

    with tile.TileContext(nc) as tc, ExitStack() as ctx, \
            nc.allow_low_precision("bf16 matmul operands by design"):
        ep = ctx.enter_context

        const_p = ep(tc.tile_pool(name="const", bufs=1))
        xt_p = ep(tc.tile_pool(name="xt", bufs=1))      # 32KB/p
        w_p = ep(tc.tile_pool(name="w", bufs=1))        # ~24.5KB/p
        qkT_p = ep(tc.tile_pool(name="qkT", bufs=1))    # 32KB/p
        vx_p = ep(tc.tile_pool(name="vx", bufs=1))      # 16.3KB/p
        qc_p = ep(tc.tile_pool(name="qc", bufs=7))      # 7 x 2KB/p
        st_p = ep(tc.tile_pool(name="st", bufs=2))      # small stats tiles
        tm_p = ep(tc.tile_pool(name="tm", bufs=5))      # 5 x 1KB/p
        pg_p = ep(tc.tile_pool(name="pg", bufs=4))      # 4 x 2KB/p
        oT_p = ep(tc.tile_pool(name="oT", bufs=1))      # 16KB/p
        scr_p = ep(tc.tile_pool(name="scr", bufs=4))
        wpm_p = ep(tc.tile_pool(name="wpm", bufs=1))    # 8KB/p
        # ps_big ([128,1024] x2 = 4 banks) is EXCLUSIVELY the S psums so the
        # S->exp double-buffering never waits on filler work; chunks,
        # transposes, recip-broadcasts and proj all rotate through ps_msc
        # ([128,512] x2 = 2 banks).
        ps_big = ep(tc.tile_pool(name="psb", bufs=2, space="PSUM"))  # 4 banks
        ps_pv = ep(tc.tile_pool(name="pspv", bufs=2, space="PSUM"))  # 2 banks
        ps_msc = ep(tc.tile_pool(name="psmsc", bufs=2, space="PSUM"))  # 2 banks

        for rep in range(reps):
            # ---- constants (DMAs deferred until after the k-path loads) ----
            _const_dmas = []

            def cst(shape, dtype, tag, src):
                t = const_p.tile(shape, dtype, tag=tag, name=f"r{rep}{tag}")
                _const_dmas.append((t, src))
                return t

            def emit_consts():
                for t, src in _const_dmas:
                    nc.sync.dma_start(t[:], src)
                _const_dmas.clear()

            ones_sb = cst([1, 128], BF, "onescol", onescol)
            id_sb = cst([128, 128], BF, "ident", ident)
            bq_sb = cst([1, 512], BF, "bqr", bqk[0:1, :])
            bk_sb = cst([1, 512], BF, "bkr", bqk[1:2, :])
            gq_sb = cst([128, 1], F32, "gqc", gqc)
            gk_sb = cst([128, 1], F32, "gkc", gkc)
            ones1_sb = cst([1, 128], F32R, "ones1", ones1)
            eps_sb = cst([128, 1], F32, "epsc", epsc)
            bp_sb = cst([128, 8], F32, "bp", bp)

            # DMA order matters: k-path inputs (wk + x band0) first so the
            # prologue's first matmuls aren't queued behind wq/wv/late bands
            wsb = {}
            _wdrams = {"k": wk, "q": wq, "v": wv}

            def load_w(wname):
                wt = w_p.tile([128, 8 * 512], BF, tag=f"w{wname}",
                              name=f"r{rep}w{wname}")
                nc.sync.dma_start(
                    wt[:].rearrange("p (kc c) -> p kc c", kc=8),
                    _wdrams[wname].rearrange("(kc p) c -> p kc c", p=128))
                wsb[wname] = [wt[:, kc * 512:(kc + 1) * 512]
                              for kc in range(8)]

            xt_all = xt_p.tile([128, 8 * SEQ], BF, tag="xt",
                               name=f"r{rep}xt")
            xt_sb = [xt_all[:, kc * SEQ:(kc + 1) * SEQ] for kc in range(8)]

            def load_band(lo, hi):
                nc.sync.dma_start(
                    xt_all[:].rearrange("p (kc t) -> p kc t", kc=8)
                    [:, :, lo:hi],
                    xT.rearrange("(kc p) t -> p kc t", p=128)[:, :, lo:hi])

            load_w("k")
            load_band(0, 128)       # unblocks k0 asap
            emit_consts()
            load_band(128, 512)
            load_w("q")
            load_band(512, 1024)
            load_w("v")
            load_band(1024, 1536)
            load_band(1536, 2048)

            # persistent destination tiles
            qnT = {}
            for hp in range(NHP):
                for wname in ("q", "k"):
                    qnT[hp, wname] = qkT_p.tile(
                        [128, SEQ], BF, tag=f"{wname}T{hp}",
                        name=f"r{rep}{wname}T{hp}")
            # vx_all: per hp 16 kc-chunks of (64 v-cols + ones col) per head
            vx_all = vx_p.tile([128, NHP * 16 * 130], BF, tag="vx",
                               name=f"r{rep}vx")
            # ones columns only (col 64 of every 65-block)
            nc.vector.memset(
                vx_all[:].rearrange("p (b s) -> p b s", s=65)[:, :, 64:65],
                1.0)

            def vx_sl(hp, kc, h):
                base = hp * 2080 + kc * 130 + 65 * h
                return vx_all[:, base:base + 65]

            oTs = {}
            for hp in range(NHP):
                oTs[hp] = oT_p.tile([128, SEQ], BF, tag=f"oT{hp}",
                                    name=f"r{rep}oT{hp}")

            # ---- qkv chunk machinery ----
            ps_chunks = {}     # (w, t16) -> psum tile
            qc_tiles = {}      # (w, t16) -> centered values, f32
            sq_tiles = {}      # (w, g4) -> [128, 32] grouped sum-of-squares
            mu_tiles = {}

            def chunk_mm(wname, t16):
                """Projection matmuls for tok-chunk t16 (PE part)."""
                ts = slice(t16 * 128, (t16 + 1) * 128)
                ps = ps_msc.tile([128, 512], F32, tag="msc",
                                 name=f"r{rep}ps{wname}{t16}")
                if with_bias and wname != "v":
                    b_sb = bq_sb if wname == "q" else bk_sb
                    nc.tensor.matmul(ps[:], lhsT=ones_sb[:],
                                     rhs=b_sb[:], start=True, stop=False)
                for kc in range(8):
                    nc.tensor.matmul(
                        ps[:],
                        lhsT=xt_sb[kc][:, ts],
                        rhs=wsb[wname][kc][:],
                        start=(kc == 0 and not (with_bias and wname != "v")),
                        stop=(kc == 7),
                    )
                ps_chunks[wname, t16] = ps

            def qk_sub(wname, t16):
                """Per-head sums + centered values tdif = 64*q - sums (DVE).
                The 64 scaling folds into gamma (g/64) at transpose evac."""
                ps = ps_chunks[wname, t16]
                qb3 = ps[:].rearrange("p (g d) -> p g d", d=D)
                mus = st_p.tile([128, 8], F32, tag="mus", bufs=4,
                                name=f"r{rep}mus{wname}{t16}")
                nc.vector.tensor_reduce(mus[:], qb3, AX.X, ALU.add)
                qc = qc_p.tile([128, 512], F32, tag="qc",
                               name=f"r{rep}qc{wname}{t16}")
                mus_v = mus[:].unsqueeze(-1).broadcast_to([128, 8, D])
                nc.vector.scalar_tensor_tensor(
                    qc[:].rearrange("p (g d) -> p g d", d=D),
                    qb3, float(D), mus_v,
                    op0=ALU.mult, op1=ALU.subtract)
                qc_tiles[wname, t16] = qc
                mu_tiles[wname, t16] = mus

            def qk_sq(wname, t16, on_act):
                """qc^2 -> q2; grouped reduce into the group's sq tile."""
                qc = qc_tiles[wname, t16]
                g4, j = t16 // 4, t16 % 4
                if (wname, g4) not in sq_tiles:
                    sq_tiles[wname, g4] = st_p.tile(
                        [128, 32], F32, tag="sq", bufs=3,
                        name=f"r{rep}sq{wname}{g4}")
                q2 = scr_p.tile([128, 512], F32, tag="q2", bufs=3,
                                name=f"r{rep}q2{wname}{t16}")
                if on_act:
                    nc.scalar.square(q2[:], qc[:])
                else:
                    nc.vector.tensor_mul(q2[:], qc[:], qc[:])
                return q2

            def qk_red(wname, t16, q2):
                g4, j = t16 // 4, t16 % 4
                nc.vector.tensor_reduce(
                    sq_tiles[wname, g4][:, 8 * j:8 * j + 8],
                    q2[:].rearrange("p (g d) -> p g d", d=D),
                    AX.X, ALU.add)

            def qk_stats(wname, t16, on_act=False):
                qk_sub(wname, t16)
                qk_red(wname, t16, qk_sq(wname, t16, on_act))

            def tr_group(wname, g4, evac_act=False):
                """Finalize tok-chunks 4*g4..4*g4+3 of q|k: batched rstd,
                apply (DVE), PE-transpose into qnT, evac applies gamma."""
                g_sb = gq_sb if wname == "q" else gk_sb
                sq = sq_tiles.pop((wname, g4))
                lnv = st_p.tile([128, 32], F32, tag="lnv", bufs=2,
                                name=f"r{rep}lnv{wname}{g4}")
                nc.scalar.activation(lnv[:], sq[:], AF.Ln,
                                     bias=eps_sb[:, 0:1],
                                     scale=1.0 / (D * D * D))
                rstd = st_p.tile([128, 32], F32, tag="rstd", bufs=2,
                                 name=f"r{rep}rstd{wname}{g4}")
                nc.scalar.activation(rstd[:], lnv[:], AF.Exp, scale=-0.5)
                tms = []
                for j in range(4):
                    t16 = 4 * g4 + j
                    qc = qc_tiles.pop((wname, t16))
                    mu_tiles.pop((wname, t16), None)
                    rstd_v = (rstd[:, 8 * j:8 * j + 8]
                              .unsqueeze(-1).broadcast_to([128, 8, D]))
                    tm = tm_p.tile([128, 512], BF, tag="tm",
                                   name=f"r{rep}tm{wname}{t16}")
                    nc.vector.tensor_mul(
                        tm[:].rearrange("p (g d) -> p g d", d=D),
                        qc[:].rearrange("p (g d) -> p g d", d=D), rstd_v)
                    tms.append(tm)
                for hp in range(NHP):
                    pstf = ps_msc.tile([128, 512], F32, tag="msc",
                                       name=f"r{rep}tr{wname}{g4}{hp}")
                    pst = pstf[:].bitcast(BF)[:, 0:512]
                    for j in range(4):
                        nc.tensor.transpose(
                            pst[:, j * 128:(j + 1) * 128],
                            tms[j][:, hp * 128:(hp + 1) * 128], id_sb[:])
                    dst = qnT[hp, wname][:, g4 * 512:(g4 + 1) * 512]
                    if evac_act:
                        nc.scalar.activation(dst, pst[:], AF.Copy,
                                             scale=g_sb[:, 0:1])
                    else:
                        nc.vector.tensor_scalar_mul(dst, pst[:],
                                                    g_sb[:, 0:1])

            def v_copy(t16, on_act=False):
                """V psum -> vx_all slices, one strided copy (all hp)."""
                ps = ps_chunks["v", t16]
                # dst: [128, hp, 2, 64] = data cols of the two 65-blocks
                dst = (vx_all[:]
                       .rearrange("p (hp kc s) -> p hp kc s", hp=NHP, kc=16)
                       [:, :, t16, :]
                       .rearrange("p hp (two s) -> p hp two s", s=65)
                       [:, :, :, 0:64])
                src = ps[:].rearrange(
                    "p (hp two s) -> p hp two s", hp=NHP, s=64)
                if on_act:
                    nc.scalar.copy(dst, src)
                else:
                    nc.vector.tensor_copy(dst, src)

            # ---- attention pipeline ----
            pvs_all = {}
            ps_tiles = {}

            def emit_s(hp, qb_i, g, h):
                qT, kT = qnT[hp, "q"], qnT[hp, "k"]
                qs = slice(qb_i * 512, (qb_i + 1) * 512)
                ps_s = ps_big.tile([128, 1024], F32, tag="big",
                                   name=f"r{rep}pss{hp}{qb_i}{g}{h}")
                for j in range(2):
                    kc = 2 * g + j
                    nc.tensor.matmul(
                        ps_s[:, j * 512:(j + 1) * 512],
                        lhsT=kT[slice(64 * h, 64 * h + 64),
                                kc * 128:(kc + 1) * 128],
                        rhs=qT[slice(64 * h, 64 * h + 64), qs],
                        start=True, stop=True,
                    )
                ps_tiles[hp, qb_i, g, h] = ps_s

            def emit_exp(hp, qb_i, g, h):
                ps_s = ps_tiles.pop((hp, qb_i, g, h))
                pg = pg_p.tile([128, 1024], BF, tag="pg",
                               name=f"r{rep}pg{hp}{qb_i}{g}{h}")
                nc.scalar.activation(pg[:], ps_s[:], AF.Exp, scale=SCALE)
                return pg

            def emit_pv(pg, hp, qb_i, g, h):
                pvs = pvs_all[hp, qb_i]
                for j in range(2):
                    kc = 2 * g + j
                    nc.tensor.matmul(
                        pvs[h][0:65, :],
                        lhsT=vx_sl(hp, kc, h),
                        rhs=pg[:, j * 512:(j + 1) * 512],
                        start=(kc == 0), stop=(kc == 15),
                    )

            if ab is not None:
                sink = scr_p.tile([1, 16], F32, tag="sink", bufs=1,
                                  name=f"r{rep}sink")
                nc.vector.memset(sink[:], 0.0)

            def _sink(ap):
                nc.vector.tensor_add(sink[:], sink[:], ap)

            def emit_divide_a(hp, qb_i):
                """Stage A at block end: evacuate PV psums on Act (frees the
                pv banks for the next block), batched fast-reciprocal of both
                denominator rows (DVE custom op; HW InstReciprocal costs
                ~2.4us), bf16 cast for the broadcast matmul."""
                pvs = pvs_all.pop((hp, qb_i))
                ocps = []
                rhbs = []
                for h in range(2):
                    ocp = scr_p.tile([128, 512], F32, tag="ocp", bufs=3,
                                     name=f"r{rep}ocp{hp}{qb_i}{h}")
                    nc.scalar.copy(ocp[0:64, :], pvs[h][0:64, :])
                    rhb = scr_p.tile([1, 512], BF, tag="rhb", bufs=3,
                                     name=f"r{rep}rhb{hp}{qb_i}{h}")
                    if _RECIP_EXACT:
                        rcp = scr_p.tile([1, 512], F32, tag="rcp", bufs=3,
                                         name=f"r{rep}rcp{hp}{qb_i}{h}")
                        nc.vector.reciprocal(rcp[:].bitcast(F32R),
                                             pvs[h][64:65, :])
                        nc.vector.tensor_copy(rhb[:], rcp[:])
                    else:
                        # 1/x = exp(-ln(x)) on Act: two tiny [1,512] ops from
                        # the already-loaded table set (HW InstReciprocal on
                        # DVE costs ~2.4us; the custom-op approx is broken on
                        # HW). The Exp writes the bf16 broadcast operand
                        # directly.
                        lnd = scr_p.tile([1, 512], F32, tag="lnd", bufs=3,
                                         name=f"r{rep}lnd{hp}{qb_i}{h}")
                        nc.scalar.activation(lnd[:], pvs[h][64:65, :],
                                             AF.Ln)
                        nc.scalar.activation(rhb[:], lnd[:], AF.Exp,
                                             scale=-1.0)
                    ocps.append(ocp)
                    rhbs.append(rhb)
                return (hp, qb_i, ocps, rhbs)

            def emit_divide_b(hp, qb_i, ocps, rhbs):
                """Stage B one iteration later (deps long fired): bf16
                ones-matmul broadcast of 1/denom, divide-mul into oT."""
                oT = oTs[hp]
                qs = slice(qb_i * 512, (qb_i + 1) * 512)
                for h in range(2):
                    psr = ps_msc.tile([128, 512], F32, tag="msc",
                                      name=f"r{rep}psrb{hp}{qb_i}{h}")
                    nc.tensor.matmul(
                        psr[0:64, :],
                        lhsT=ones_sb[0:1, 0:64], rhs=rhbs[h][:],
                        start=True, stop=True,
                    )
                    nc.vector.tensor_mul(
                        oT[64 * h:64 * h + 64, qs],
                        ocps[h][0:64, :], psr[0:64, :])

            # ---- projection ----
            wpm_all = wpm_p.tile([128, 8 * 4 * 128], BF, tag="wpm",
                                 name=f"r{rep}wpmall")

            def load_wpm():
                nc.sync.dma_start(
                    wpm_all[:].rearrange("p (m hp c) -> p m hp c",
                                         m=8, hp=4),
                    wp.rearrange("(hp p) (m c) -> p m hp c", p=128, c=128))

            def proj_group(m, n):
                ps = ps_msc.tile([128, 512], F32, tag="msc",
                                 name=f"r{rep}pspr{m}{n}")
                for hp in range(NHP):
                    nc.tensor.matmul(
                        ps[:],
                        lhsT=wpm_all[:, (m * 4 + hp) * 128:
                                     (m * 4 + hp + 1) * 128],
                        rhs=oTs[hp][:, n * 512:(n + 1) * 512],
                        start=(hp == 0), stop=(hp == NHP - 1),
                    )
                so = scr_p.tile([128, 512], F32, tag="so", bufs=4,
                                name=f"r{rep}so{m}{n}")
                nc.scalar.activation(so[:], ps[:], AF.Identity,
                                     bias=bp_sb[:, m:m + 1])
                nc.sync.dma_start(
                    outT[m * 128:(m + 1) * 128,
                         n * 512:(n + 1) * 512], so[:])

            # ---- prologue: k0-5, tr_k(0), q0-3, tr_q(0), v0-1 ----
            if ab is None:
                for t16 in range(4):
                    chunk_mm("k", t16)
                    qk_stats("k", t16)
                tr_group("k", 0, evac_act=True)
                for t16 in range(4):
                    chunk_mm("q", t16)
                    qk_stats("q", t16)
                tr_group("q", 0, evac_act=True)
                for t16 in (4, 5):
                    chunk_mm("k", t16)
                    qk_stats("k", t16)
                chunk_mm("v", 0)
                v_copy(0, on_act=True)
                chunk_mm("v", 1)
                v_copy(1, on_act=True)
            else:
                # ablation: fake qnT/vx contents, skip all LN machinery
                for hp in range(NHP):
                    for wname in ("q", "k"):
                        nc.vector.memset(qnT[hp, wname][:], 0.05)
                nc.vector.memset(vx_all[:], 0.01)
                load_wpm()

            # ---- 256 attention steps, qb-major blocks ----
            steps = [(hp, qb_i, g, h)
                     for qb_i in range(4) for hp in range(NHP)
                     for g in range(8) for h in range(2)]
            fill_pre = {}    # PE-heavy fillers: run between S and exp
            fill_post = {}   # DVE/Act fillers: run after exp+PV

            def pre(i, fn):
                fill_pre.setdefault(i, []).append(fn)

            def post(i, fn):
                fill_post.setdefault(i, []).append(fn)

            # k chunks 6..15 stats pipelined: mm@s, sub@s, sq(Act)@s+1,
            # red@s+2.  tr_k(gi) needs chunks 4gi..4gi+3 reduced by step 4gi.
            q2_tiles = {}
            for idx, t16 in enumerate(range(6, 16) if ab is None else ()):
                s = idx  # steps 0..9
                pre(s, lambda t=t16: chunk_mm("k", t))
                post(s, lambda t=t16: qk_sub("k", t))
                post(s + 1, lambda t=t16: q2_tiles.__setitem__(
                    t, qk_sq("k", t, True)))
                post(s + 2, lambda t=t16: qk_red(
                    "k", t, q2_tiles.pop(t)))
            if ab is None:
                post(3, lambda: tr_group("k", 1, evac_act=True))
                post(7, lambda: tr_group("k", 2, evac_act=True))
                post(11, lambda: tr_group("k", 3, evac_act=True))

            # v chunks 2..15: needed by PV at iteration t+2
            for t16 in (range(2, 16) if ab is None else ()):
                s = t16 - 2  # steps 0..13
                pre(s, lambda t=t16: chunk_mm("v", t))
                post(s, lambda t=t16: v_copy(t, on_act=True))

            # q chunks 4..15, pipelined like k; groups due at 64*g4
            for g4 in (range(1, 4) if ab is None else ()):
                base = 16 * g4
                for j in range(4):
                    t16 = 4 * g4 + j
                    pre(base + 3 * j, lambda t=t16: chunk_mm("q", t))
                    post(base + 3 * j, lambda t=t16: qk_sub("q", t))
                    post(base + 3 * j + 1, lambda t=t16: q2_tiles.__setitem__(
                        ("q", t), qk_sq("q", t, True)))
                    post(base + 3 * j + 2, lambda t=t16: qk_red(
                        "q", t, q2_tiles.pop(("q", t))))
                post(base + 12, lambda g=g4: tr_group("q", g, evac_act=True))

            if ab is None:
                pre(40, load_wpm)

            # Pipeline: S@i, exp@i-1, PV@i-2 -- by the time PV(i-2) and
            # S(i) reach the PE FIFO their sems have already fired, so the
            # PE never stalls mid-stream and the exp stream runs at the Act
            # engine's native rate.
            proj_pending = []
            pend_exp = None          # step awaiting exp
            pend_pv = None           # (pg, step) awaiting PV
            pend_div = []            # divide stage-B args, run next iteration

            def do_pv(pg, st):
                emit_pv(pg, *st)
                if st[2:] == (7, 1) and ab != "pipe":
                    php, pqb = st[:2]
                    pend_div.append(emit_divide_a(php, pqb))
                    if php == NHP - 1 and pqb < 3 and ab in (None, "proj"):
                        for m in range(8):
                            proj_pending.append(
                                lambda m=m, n=pqb: proj_group(m, n))

            for i, st in enumerate(steps):
                hp, qb_i, g, h = st
                if (g, h) == (0, 0):
                    pvs_all[hp, qb_i] = [
                        ps_pv.tile([128, 512], F32, tag="pv",
                                   name=f"r{rep}pv{hp}{qb_i}{_h}")
                        for _h in range(2)]
                emit_s(*st)
                if pend_div:
                    emit_divide_b(*pend_div.pop(0))
                for t in fill_pre.get(i, ()):
                    t()
                if pend_pv is not None:
                    do_pv(*pend_pv)
                    pend_pv = None
                if pend_exp is not None:
                    pend_pv = (emit_exp(*pend_exp), pend_exp)
                pend_exp = st
                for t in fill_post.get(i, ()):
                    t()
                if proj_pending and i % 2 == 0:
                    proj_pending.pop(0)()
            # drain
            last_pg = emit_exp(*pend_exp)
            do_pv(*pend_pv)
            do_pv(last_pg, pend_exp)
            for d in pend_div:
                emit_divide_b(*d)
            for t in proj_pending:
                t()
            if ab in (None, "proj"):
                for m in range(8):
                    proj_group(m, 3)
            elif ab == "pipe":
                for h in range(2):
                    sod = scr_p.tile([128, 512], F32, tag="so", bufs=4,
                                     name=f"r{rep}abl{h}")
                    nc.vector.tensor_copy(sod[:], pvs_all[3, 3][h][:])
                    nc.sync.dma_start(outT[128 * h:128 * (h + 1), 0:512],
                                      sod[:])
            elif ab == "div":
                if _DIVMODE == "full":
                    for hp in range(NHP):
                        sod = scr_p.tile([128, 512], F32, tag="so", bufs=4,
                                         name=f"r{rep}abl{hp}")
                        nc.vector.tensor_copy(sod[:], oTs[hp][:, 0:512])
                        nc.sync.dma_start(
                            outT[128 * hp:128 * (hp + 1), 0:512], sod[:])
                else:
                    nc.sync.dma_start(outT[0:1, 0:16], sink[:])

    nc.compile()
    return nc


def make_in_maps(x, w_qkv, b_qkv, g_q, g_k, w_proj, b_proj):
    """Host-side sharding: per-core input dict (weights pre-cast to bf16)."""
    import ml_dtypes
    f32 = np.float32
    bf16 = ml_dtypes.bfloat16
    x = np.ascontiguousarray(x, dtype=f32)
    w_qkv = np.asarray(w_qkv, dtype=f32)
    b_qkv = np.asarray(b_qkv, dtype=f32)
    g_q = np.asarray(g_q, dtype=f32)
    g_k = np.asarray(g_k, dtype=f32)
    w_proj = np.asarray(w_proj, dtype=f32)
    b_proj = np.asarray(b_proj, dtype=f32)

    ident = np.eye(128, dtype=f32)
    onescol = np.ones((1, 128), f32)
    ones1 = np.ones((1, 128), f32)
    # g/64 per d-row of a head pair (folds the tdif = 64*(q-mu) scaling)
    gqc = np.concatenate([g_q, g_q]).reshape(128, 1) * (1.0 / 64.0)
    gkc = np.concatenate([g_k, g_k]).reshape(128, 1) * (1.0 / 64.0)

    in_maps = []
    for c in range(NCORES):
        b = c // 2
        hg = c % 2
        cs = slice(hg * 512, (hg + 1) * 512)
        bv = b_qkv[2 * C:][cs]
        wp_half = w_proj[hg * 512:(hg + 1) * 512, :]
        bp_eff = bv @ wp_half + (b_proj if hg == 0 else 0.0)
        bqk_rows = np.stack([b_qkv[cs], b_qkv[C:][cs]])
        in_maps.append({
            "xT": np.ascontiguousarray(x[b].T).astype(bf16),
            "wq": np.ascontiguousarray(w_qkv[:, cs]).astype(bf16),
            "wk": np.ascontiguousarray(w_qkv[:, C:][:, cs]).astype(bf16),
            "wv": np.ascontiguousarray(w_qkv[:, 2 * C:][:, cs]).astype(bf16),
            "wp": np.ascontiguousarray(wp_half).astype(bf16),
            "bqk": bqk_rows.astype(bf16),
            "onescol": onescol.astype(bf16),
            "ident": ident.astype(bf16),
            "gqc": gqc,
            "gkc": gkc,
            "ones1": ones1,
            "epsc": np.full((128, 1), EPS, f32),
            "bp": np.ascontiguousarray(bp_eff.reshape(8, 128).T.astype(f32)),
        })
    return in_maps


def unshard(partials):
    """partials: list of 8 outT arrays [C, SEQ] -> full [B, SEQ, C]."""
    out = np.empty((B, SEQ, C), np.float32)
    for b in range(B):
        out[b] = (partials[2 * b] + partials[2 * b + 1]).T
    return out


def kernel(x, w_qkv, b_qkv, g_q, g_k, w_proj, b_proj):
    from concourse.bass_utils import run_bass_kernel_spmd

    # q/k biases feed the pre-LN values; emit the bias-prefill matmuls only
    # if they are actually nonzero (the spec fills b_qkv with zeros)
    wb = bool(np.any(np.asarray(b_qkv)[:2 * C]))
    key = ("nc", wb)
    if key not in _CACHE:
        _CACHE[key] = _build_nc(with_bias=wb)
    nc = _CACHE[key]
    in_maps = make_in_maps(x, w_qkv, b_qkv, g_q, g_k, w_proj, b_proj)
    res = run_bass_kernel_spmd(nc, in_maps, list(range(NCORES)))
    return unshard([res.results[c]["outT"] for c in range(NCORES)])


# revision 39
# speedup vs baseline: 1.5118x; 1.3100x over previous
"""Trainium2 Bass kernel for nn_AttentionQKNorm (B=4, N=2048, C=1024, H=16, D=64).

Sharding (8 cores): core c -> batch b = c//2, head-group hg = c%2 (8 heads).
Tensor-parallel within a batch: each core computes qkv for its 8 heads,
per-head QK-LayerNorm, attention, and a partial projection
o_part @ w_proj[rows] -> [2048, 1024]. Host sums the two partials per batch.

v3 design, tuned against HW microbenchmarks (model rates are wrong on
real TRN2: N=512 bf16 matmuls ~131ns not 213; exp[128,1024] from psum
~490ns not 1038; InstReciprocal ~2.4us not 0.6):
  - 3-stage software pipeline over the 256 attention steps: S@i (PE),
    exp@i-1 (Act), PV@i-2 (PE). With PV delayed two steps behind S,
    every op's semaphores have fired before it reaches its engine FIFO,
    so neither PE nor Act ever stalls mid-stream (measured 475ns/step
    vs 1240ns for the naive S/exp/PV order).
  - psum: ps_big ([128,1024]x2) is EXCLUSIVELY the S psums (the S->exp
    double-buffer never waits on filler work); qkv chunks, transposes,
    1/denom broadcasts and proj rotate through ps_msc ([128,512]x2);
    PV accumulators keep 2 banks.
  - LN chain per 128-token chunk: DVE reduce (sums) + stt
    (tdif = 64q - sums), Act Square (pipelined 1 step behind), DVE
    grouped reduce, batched rstd (one Ln+Exp per 4-chunk group on Act),
    DVE apply mul; 1/64 folds into gamma at the transpose evac
    (Act Copy with per-partition scale AP).
  - V psum evac: ONE strided Act copy per chunk into vx_all (data cols
    0:64 of each 65-col block; ones cols via one strided memset).
  - divide, split in two stages: at block end Act evacuates PV psum
    rows 0:64 (frees the pv banks) and computes 1/denom as
    Exp(-Ln(denom)) straight from the psum denominator row (two tiny
    Act ops from the already-loaded table set, writing bf16); one
    iteration later (deps fired) a bf16 ones-matmul broadcasts it and
    a DVE mul writes oT. The exact DVE reciprocal costs ~2.4us/call on
    HW and serialized the whole pipeline (+400us); this chain is ~1us
    off the critical path.
  - blocks are qb-major so projection columns complete early and the
    proj work spreads over blocks 5..16 instead of a serial tail.

All matmul operands bf16 (host pre-converts); f32 psum accumulation.
"""

import numpy as np

H = 16
D = 64
B = 4
SEQ = 2048
C = 1024
NCORES = 8
NHP = 4  # head-pairs per core (8 heads)
EPS = 1e-6
SCALE = D**-0.5

_CACHE = {}

# timing-ablation switch (used by ablate.py only):
#   None: full kernel; "pipe": S/exp/PV only; "div": +divide; "proj": +proj
_ABLATE = None
# divide-chain ablation: "full" | "ocp" | "recip" | "psr" (cumulative stages)
_DIVMODE = "full"
_RECIP_EXACT = False


def _build_nc(reps=1, with_bias=False):
    ab = _ABLATE
    from contextlib import ExitStack

    import concourse.bacc as bacc
    import concourse.tile as tile
    import concourse.mybir as mybir

    dt = mybir.dt
    F32, F32R, BF = dt.float32, dt.float32r, dt.bfloat16
    AF = mybir.ActivationFunctionType
    ALU = mybir.AluOpType
    AX = mybir.AxisListType

    # Ln and Exp both live in act-table set "natural_log_exp_and_others"
    # (which also has Square/Copy/Identity), but the table-load inserter
    # picks the FIRST set containing each func, causing a ~2.7us table
    # reload around every LayerNorm rstd. Hide Ln/Exp from the earlier
    # sets so both resolve to the shared set -> one load total.
    if not getattr(bacc, "_qknorm_act_tables_patched", False):
        _orig_get_tables = bacc.get_activation_tables

        def _patched_get_tables(arch):
            tabs = {k: set(v) for k, v in _orig_get_tables(arch).items()}
            af = mybir.ActivationFunctionType
            both = "natural_log_exp_and_others"
            if both in tabs and af.Exp in tabs[both] and af.Ln in tabs[both]:
                for name, funcs in tabs.items():
                    if name != both:
                        funcs.discard(af.Ln)
                        funcs.discard(af.Exp)
            return tabs

        bacc.get_activation_tables = _patched_get_tables
        bacc._qknorm_act_tables_patched = True

    nc = bacc.Bacc("TRN2", target_bir_lowering=False, debug=False,
                   num_devices=NCORES)

    def din(name, shape, dtype=BF):
        return nc.dram_tensor(name, shape, dtype, kind="ExternalInput").ap()

    xT = din("xT", [C, SEQ])
    wq = din("wq", [C, 512])
    wk = din("wk", [C, 512])
    wv = din("wv", [C, 512])
    wp = din("wp", [512, C])
    bqk = din("bqk", [2, 512])          # rows: bq, bk (this half)
    onescol = din("onescol", [1, 128])  # bias-prefill lhsT
    ident = din("ident", [128, 128])    # transpose rhs
    gqc = din("gqc", [128, 1], F32)     # g_q per d-row of a head pair
    gkc = din("gkc", [128, 1], F32)
    ones1 = din("ones1", [1, 128], F32R)
    epsc = din("epsc", [128, 1], F32)
    bp = din("bp", [128, 8], F32)       # b_proj + bv@wp, [128, m]
    outT = nc.dram_tensor("outT", [C, SEQ], F32, kind="ExternalOutput").ap()

    with tile.TileContext(nc) as tc, ExitStack() as ctx, \
            nc.allow_low_precision("bf16 matmul operands by design"):
        ep = ctx.enter_context

        const_p = ep(tc.tile_pool(name="const", bufs=1))
        xt_p = ep(tc.tile_pool(name="xt", bufs=1))      # 32KB/p
        w_p = ep(tc.tile_pool(name="w", bufs=1))        # ~24.5KB/p
        qkT_p = ep(tc.tile_pool(name="qkT", bufs=1))    # 32KB/p
        vx_p = ep(tc.tile_pool(name="vx", bufs=1))      # 16.3KB/p
        qc_p = ep(tc.tile_pool(name="qc", bufs=7))      # 7 x 2KB/p
        st_p = ep(tc.tile_pool(name="st", bufs=2))      # small stats tiles
        tm_p = ep(tc.tile_pool(name="tm", bufs=5))      # 5 x 1KB/p
        pg_p = ep(tc.tile_pool(name="pg", bufs=4))      # 4 x 2KB/p
        oT_p = ep(tc.tile_pool(name="oT", bufs=1))      # 16KB/p
        scr_p = ep(tc.tile_pool(name="scr", bufs=4))
        wpm_p = ep(tc.tile_pool(name="wpm", bufs=1))    # 8KB/p
        # ps_big ([128,1024] x2 = 4 banks) is EXCLUSIVELY the S psums so the
        # S->exp double-buffering never waits on filler work; chunks,
        # transposes, recip-broadcasts and proj all rotate through ps_msc
        # ([128,512] x2 = 2 banks).
        ps_big = ep(tc.tile_pool(name="psb", bufs=2, space="PSUM"))  # 4 banks
        ps_pv = ep(tc.tile_pool(name="pspv", bufs=2, space="PSUM"))  # 2 banks
        ps_msc = ep(tc.tile_pool(name="psmsc", bufs=2, space="PSUM"))  # 2 banks

        for rep in range(reps):
            # ---- constants (DMAs deferred until after the k-path loads) ----
            _const_dmas = []

            def cst(shape, dtype, tag, src):
                t = const_p.tile(shape, dtype, tag=tag, name=f"r{rep}{tag}")
                _const_dmas.append((t, src))
                return t

            def emit_consts():
                for t, src in _const_dmas:
                    nc.sync.dma_start(t[:], src)
                _const_dmas.clear()

            ones_sb = cst([1, 128], BF, "onescol", onescol)
            id_sb = cst([128, 128], BF, "ident", ident)
            bq_sb = cst([1, 512], BF, "bqr", bqk[0:1, :])
            bk_sb = cst([1, 512], BF, "bkr", bqk[1:2, :])
            gq_sb = cst([128, 1], F32, "gqc", gqc)
            gk_sb = cst([128, 1], F32, "gkc", gkc)
            ones1_sb = cst([1, 128], F32R, "ones1", ones1)
            eps_sb = cst([128, 1], F32, "epsc", epsc)
            bp_sb = cst([128, 8], F32, "bp", bp)

            # DMA order matters: k-path inputs (wk + x band0) first so the
            # prologue's first matmuls aren't queued behind wq/wv/late bands
            wsb = {}
            _wdrams = {"k": wk, "q": wq, "v": wv}

            def load_w(wname):
                wt = w_p.tile([128, 8 * 512], BF, tag=f"w{wname}",
                              name=f"r{rep}w{wname}")
                nc.sync.dma_start(
                    wt[:].rearrange("p (kc c) -> p kc c", kc=8),
                    _wdrams[wname].rearrange("(kc p) c -> p kc c", p=128))
                wsb[wname] = [wt[:, kc * 512:(kc + 1) * 512]
                              for kc in range(8)]

            xt_all = xt_p.tile([128, 8 * SEQ], BF, tag="xt",
                               name=f"r{rep}xt")
            xt_sb = [xt_all[:, kc * SEQ:(kc + 1) * SEQ] for kc in range(8)]

            def load_band(lo, hi):
                nc.sync.dma_start(
                    xt_all[:].rearrange("p (kc t) -> p kc t", kc=8)
                    [:, :, lo:hi],
                    xT.rearrange("(kc p) t -> p kc t", p=128)[:, :, lo:hi])

            load_w("k")
            load_band(0, 128)       # unblocks k0 asap
            emit_consts()
            load_band(128, 512)
            load_w("q")
            load_band(512, 1024)
            load_w("v")
            load_band(1024, 1536)
            load_band(1536, 2048)

            # persistent destination tiles
            qnT = {}
            for hp in range(NHP):
                for wname in ("q", "k"):
                    qnT[hp, wname] = qkT_p.tile(
                        [128, SEQ], BF, tag=f"{wname}T{hp}",
                        name=f"r{rep}{wname}T{hp}")
            # vx_all: per hp 16 kc-chunks of (64 v-cols + ones col) per head
            vx_all = vx_p.tile([128, NHP * 16 * 130], BF, tag="vx",
                               name=f"r{rep}vx")
            # ones columns only (col 64 of every 65-block)
            nc.vector.memset(
                vx_all[:].rearrange("p (b s) -> p b s", s=65)[:, :, 64:65],
                1.0)

            def vx_sl(hp, kc, h):
                base = hp * 2080 + kc * 130 + 65 * h
                return vx_all[:, base:base + 65]

            oTs = {}
            for hp in range(NHP):
                oTs[hp] = oT_p.tile([128, SEQ], BF, tag=f"oT{hp}",
                                    name=f"r{rep}oT{hp}")

            # ---- qkv chunk machinery ----
            ps_chunks = {}     # (w, t16) -> psum tile
            qc_tiles = {}      # (w, t16) -> centered values, f32
            sq_tiles = {}      # (w, g4) -> [128, 32] grouped sum-of-squares
            mu_tiles = {}

            def chunk_mm(wname, t16):
                """Projection matmuls for tok-chunk t16 (PE part)."""
                ts = slice(t16 * 128, (t16 + 1) * 128)
                ps = ps_msc.tile([128, 512], F32, tag="msc",
                                 name=f"r{rep}ps{wname}{t16}")
                if with_bias and wname != "v":
                    b_sb = bq_sb if wname == "q" else bk_sb
                    nc.tensor.matmul(ps[:], lhsT=ones_sb[:],
                                     rhs=b_sb[:], start=True, stop=False)
                for kc in range(8):
                    nc.tensor.matmul(
                        ps[:],
                        lhsT=xt_sb[kc][:, ts],
                        rhs=wsb[wname][kc][:],
                        start=(kc == 0 and not (with_bias and wname != "v")),
                        stop=(kc == 7),
                    )
                ps_chunks[wname, t16] = ps

            def qk_sub(wname, t16):
                """Per-head sums + centered values tdif = 64*q - sums (DVE).
                The 64 scaling folds into gamma (g/64) at transpose evac."""
                ps = ps_chunks[wname, t16]
                qb3 = ps[:].rearrange("p (g d) -> p g d", d=D)
                mus = st_p.tile([128, 8], F32, tag="mus", bufs=4,
                                name=f"r{rep}mus{wname}{t16}")
                nc.vector.tensor_reduce(mus[:], qb3, AX.X, ALU.add)
                qc = qc_p.tile([128, 512], F32, tag="qc",
                               name=f"r{rep}qc{wname}{t16}")
                mus_v = mus[:].unsqueeze(-1).broadcast_to([128, 8, D])
                nc.vector.scalar_tensor_tensor(
                    qc[:].rearrange("p (g d) -> p g d", d=D),
                    qb3, float(D), mus_v,
                    op0=ALU.mult, op1=ALU.subtract)
                qc_tiles[wname, t16] = qc
                mu_tiles[wname, t16] = mus

            def qk_sq(wname, t16, on_act):
                """qc^2 -> q2; grouped reduce into the group's sq tile."""
                qc = qc_tiles[wname, t16]
                g4, j = t16 // 4, t16 % 4
                if (wname, g4) not in sq_tiles:
                    sq_tiles[wname, g4] = st_p.tile(
                        [128, 32], F32, tag="sq", bufs=3,
                        name=f"r{rep}sq{wname}{g4}")
                q2 = scr_p.tile([128, 512], F32, tag="q2", bufs=3,
                                name=f"r{rep}q2{wname}{t16}")
                if on_act:
                    nc.scalar.square(q2[:], qc[:])
                else:
                    nc.vector.tensor_mul(q2[:], qc[:], qc[:])
                return q2

            def qk_red(wname, t16, q2):
                g4, j = t16 // 4, t16 % 4
                nc.vector.tensor_reduce(
                    sq_tiles[wname, g4][:, 8 * j:8 * j + 8],
                    q2[:].rearrange("p (g d) -> p g d", d=D),
                    AX.X, ALU.add)

            def qk_stats(wname, t16, on_act=False):
                qk_sub(wname, t16)
                qk_red(wname, t16, qk_sq(wname, t16, on_act))

            def tr_group(wname, g4, evac_act=False):
                """Finalize tok-chunks 4*g4..4*g4+3 of q|k: batched rstd,
                apply (DVE), PE-transpose into qnT, evac applies gamma."""
                g_sb = gq_sb if wname == "q" else gk_sb
                sq = sq_tiles.pop((wname, g4))
                lnv = st_p.tile([128, 32], F32, tag="lnv", bufs=2,
                                name=f"r{rep}lnv{wname}{g4}")
                nc.scalar.activation(lnv[:], sq[:], AF.Ln,
                                     bias=eps_sb[:, 0:1],
                                     scale=1.0 / (D * D * D))
                rstd = st_p.tile([128, 32], F32, tag="rstd", bufs=2,
                                 name=f"r{rep}rstd{wname}{g4}")
                nc.scalar.activation(rstd[:], lnv[:], AF.Exp, scale=-0.5)
                tms = []
                for j in range(4):
                    t16 = 4 * g4 + j
                    qc = qc_tiles.pop((wname, t16))
                    mu_tiles.pop((wname, t16), None)
                    rstd_v = (rstd[:, 8 * j:8 * j + 8]
                              .unsqueeze(-1).broadcast_to([128, 8, D]))
                    tm = tm_p.tile([128, 512], BF, tag="tm",
                                   name=f"r{rep}tm{wname}{t16}")
                    nc.vector.tensor_mul(
                        tm[:].rearrange("p (g d) -> p g d", d=D),
                        qc[:].rearrange("p (g d) -> p g d", d=D), rstd_v)
                    tms.append(tm)
                for hp in range(NHP):
                    pstf = ps_msc.tile([128, 512], F32, tag="msc",
                                       name=f"r{rep}tr{wname}{g4}{hp}")
                    pst = pstf[:].bitcast(BF)[:, 0:512]
                    for j in range(4):
                        nc.tensor.transpose(
                            pst[:, j * 128:(j + 1) * 128],
                            tms[j][:, hp * 128:(hp + 1) * 128], id_sb[:])
                    dst = qnT[hp, wname][:, g4 * 512:(g4 + 1) * 512]
                    if evac_act:
                        nc.scalar.activation(dst, pst[:], AF.Copy,
                                             scale=g_sb[:, 0:1])
                    else:
                        nc.vector.tensor_scalar_mul(dst, pst[:],
                                                    g_sb[:, 0:1])

            def v_copy(t16, on_act=False):
                """V psum -> vx_all slices, one strided copy (all hp)."""
                ps = ps_chunks["v", t16]
                # dst: [128, hp, 2, 64] = data cols of the two 65-blocks
                dst = (vx_all[:]
                       .rearrange("p (hp kc s) -> p hp kc s", hp=NHP, kc=16)
                       [:, :, t16, :]
                       .rearrange("p hp (two s) -> p hp two s", s=65)
                       [:, :, :, 0:64])
                src = ps[:].rearrange(
                    "p (hp two s) -> p hp two s", hp=NHP, s=64)
                if on_act:
                    nc.scalar.copy(dst, src)
                else:
                    nc.vector.tensor_copy(dst, src)

            # ---- attention pipeline ----
            pvs_all = {}
            ps_tiles = {}

            def emit_s(hp, qb_i, g, h):
                qT, kT = qnT[hp, "q"], qnT[hp, "k"]
                qs = slice(qb_i * 512, (qb_i + 1) * 512)
                ps_s = ps_big.tile([128, 1024], F32, tag="big",
                                   name=f"r{rep}pss{hp}{qb_i}{g}{h}")
                for j in range(2):
                    kc = 2 * g + j
                    nc.tensor.matmul(
                        ps_s[:, j * 512:(j + 1) * 512],
                        lhsT=kT[slice(64 * h, 64 * h + 64),
                                kc * 128:(kc + 1) * 128],
                        rhs=qT[slice(64 * h, 64 * h + 64), qs],
                        start=True, stop=True,
                    )
                ps_tiles[hp, qb_i, g, h] = ps_s

            def emit_exp(hp, qb_i, g, h):
                ps_s = ps_tiles.pop((hp, qb_i, g, h))
                pg = pg_p.tile([128, 1024], BF, tag="pg",
                               name=f"r{rep}pg{hp}{qb_i}{g}{h}")
                nc.scalar.activation(pg[:], ps_s[:], AF.Exp, scale=SCALE)
                return pg

            def emit_pv(pg, hp, qb_i, g, h):
                pvs = pvs_all[hp, qb_i]
                for j in range(2):
                    kc = 2 * g + j
                    nc.tensor.matmul(
                        pvs[h][0:65, :],
                        lhsT=vx_sl(hp, kc, h),
                        rhs=pg[:, j * 512:(j + 1) * 512],
                        start=(kc == 0), stop=(kc == 15),
                    )

            if ab is not None:
                sink = scr_p.tile([1, 16], F32, tag="sink", bufs=1,
                                  name=f"r{rep}sink")
                nc.vector.memset(sink[:], 0.0)

            def _sink(ap):
                nc.vector.tensor_add(sink[:], sink[:], ap)

            def emit_divide_a(hp, qb_i):
                """Stage A at block end: evacuate PV psums on Act (frees the
                pv banks for the next block), batched fast-reciprocal of both
                denominator rows (DVE custom op; HW InstReciprocal costs
                ~2.4us), bf16 cast for the broadcast matmul."""
                pvs = pvs_all.pop((hp, qb_i))
                ocps = []
                rhbs = []
                for h in range(2):
                    ocp = scr_p.tile([128, 512], F32, tag="ocp", bufs=3,
                                     name=f"r{rep}ocp{hp}{qb_i}{h}")
                    nc.scalar.copy(ocp[0:64, :], pvs[h][0:64, :])
                    rhb = scr_p.tile([1, 512], BF, tag="rhb", bufs=3,
                                     name=f"r{rep}rhb{hp}{qb_i}{h}")
                    if _RECIP_EXACT:
                        rcp = scr_p.tile([1, 512], F32, tag="rcp", bufs=3,
                                         name=f"r{rep}rcp{hp}{qb_i}{h}")
                        nc.vector.reciprocal(rcp[:].bitcast(F32R),
                                             pvs[h][64:65, :])
                        nc.vector.tensor_copy(rhb[:], rcp[:])
                    else:
                        # 1/x = exp(-ln(x)) on Act: two tiny [1,512] ops from
                        # the already-loaded table set (HW InstReciprocal on
                        # DVE costs ~2.4us; the custom-op approx is broken on
                        # HW). The Exp writes the bf16 broadcast operand
                        # directly.
                        lnd = scr_p.tile([1, 512], F32, tag="lnd", bufs=3,
                                         name=f"r{rep}lnd{hp}{qb_i}{h}")
                        nc.scalar.activation(lnd[:], pvs[h][64:65, :],
                                             AF.Ln)
                        nc.scalar.activation(rhb[:], lnd[:], AF.Exp,
                                             scale=-1.0)
                    ocps.append(ocp)
                    rhbs.append(rhb)
                return (hp, qb_i, ocps, rhbs)

            def emit_divide_b(hp, qb_i, ocps, rhbs):
                """Stage B one iteration later (deps long fired): bf16
                ones-matmul broadcast of 1/denom, divide-mul into oT."""
                oT = oTs[hp]
                qs = slice(qb_i * 512, (qb_i + 1) * 512)
                for h in range(2):
                    psr = ps_msc.tile([128, 512], F32, tag="msc",
                                      name=f"r{rep}psrb{hp}{qb_i}{h}")
                    nc.tensor.matmul(
                        psr[0:64, :],
                        lhsT=ones_sb[0:1, 0:64], rhs=rhbs[h][:],
                        start=True, stop=True,
                    )
                    nc.vector.tensor_mul(
                        oT[64 * h:64 * h + 64, qs],
                        ocps[h][0:64, :], psr[0:64, :])

            # ---- projection ----
            wpm_all = wpm_p.tile([128, 8 * 4 * 128], BF, tag="wpm",
                                 name=f"r{rep}wpmall")

            def load_wpm():
                nc.sync.dma_start(
                    wpm_all[:].rearrange("p (m hp c) -> p m hp c",
                                         m=8, hp=4),
                    wp.rearrange("(hp p) (m c) -> p m hp c", p=128, c=128))

            def proj_group(m, n):
                ps = ps_msc.tile([128, 512], F32, tag="msc",
                                 name=f"r{rep}pspr{m}{n}")
                for hp in range(NHP):
                    nc.tensor.matmul(
                        ps[:],
                        lhsT=wpm_all[:, (m * 4 + hp) * 128:
                                     (m * 4 + hp + 1) * 128],
                        rhs=oTs[hp][:, n * 512:(n + 1) * 512],
                        start=(hp == 0), stop=(hp == NHP - 1),
                    )
                so = scr_p.tile([128, 512], F32, tag="so", bufs=4,
                                name=f"r{rep}so{m}{n}")
                nc.scalar.activation(so[:], ps[:], AF.Identity,
                                     bias=bp_sb[:, m:m + 1])
                nc.sync.dma_start(
                    outT[m * 128:(m + 1) * 128,
                         n * 512:(n + 1) * 512], so[:])

            # ---- prologue: k0-5, tr_k(0), q0-3, tr_q(0), v0-1 ----
            if ab is None:
                for t16 in range(4):
                    chunk_mm("k", t16)
                    qk_stats("k", t16)
                tr_group("k", 0, evac_act=True)
                for t16 in range(4):
                    chunk_mm("q", t16)
                    qk_stats("q", t16)
                tr_group("q", 0, evac_act=True)
                for t16 in (4, 5):
                    chunk_mm("k", t16)
                    qk_stats("k", t16)
                chunk_mm("v", 0)
                v_copy(0, on_act=True)
                chunk_mm("v", 1)
                v_copy(1, on_act=True)
            else:
                # ablation: fake qnT/vx contents, skip all LN machinery
                for hp in range(NHP):
                    for wname in ("q", "k"):
                        nc.vector.memset(qnT[hp, wname][:], 0.05)
                nc.vector.memset(vx_all[:], 0.01)
                load_wpm()

            # ---- 256 attention steps, qb-major blocks ----
            steps = [(hp, qb_i, g, h)
                     for qb_i in range(4) for hp in range(NHP)
                     for g in range(8) for h in range(2)]
            fill_pre = {}    # PE-heavy fillers: run between S and exp
            fill_post = {}   # DVE/Act fillers: run after exp+PV

            def pre(i, fn):
                fill_pre.setdefault(i, []).append(fn)

            def post(i, fn):
                fill_post.setdefault(i, []).append(fn)

            # k chunks 6..15 stats pipelined: mm@s, sub@s, sq(Act)@s+1,
            # red@s+2.  tr_k(gi) needs chunks 4gi..4gi+3 reduced by step 4gi.
            q2_tiles = {}
            for idx, t16 in enumerate(range(6, 16) if ab is None else ()):
                s = idx  # steps 0..9
                pre(s, lambda t=t16: chunk_mm("k", t))
                post(s, lambda t=t16: qk_sub("k", t))
                post(s + 1, lambda t=t16: q2_tiles.__setitem__(
                    t, qk_sq("k", t, True)))
                post(s + 2, lambda t=t16: qk_red(
                    "k", t, q2_tiles.pop(t)))
            if ab is None:
                post(3, lambda: tr_group("k", 1, evac_act=True))
                post(7, lambda: tr_group("k", 2, evac_act=True))
                post(11, lambda: tr_group("k", 3, evac_act=True))

            # v chunks 2..15: needed by PV at iteration t+2
            for t16 in (range(2, 16) if ab is None else ()):
                s = t16 - 2  # steps 0..13
                pre(s, lambda t=t16: chunk_mm("v", t))
                post(s, lambda t=t16: v_copy(t, on_act=True))

            # q chunks 4..15, pipelined like k; groups due at 64*g4
            for g4 in (range(1, 4) if ab is None else ()):
                base = 16 * g4
                for j in range(4):
                    t16 = 4 * g4 + j
                    pre(base + 3 * j, lambda t=t16: chunk_mm("q", t))
                    post(base + 3 * j, lambda t=t16: qk_sub("q", t))
                    post(base + 3 * j + 1, lambda t=t16: q2_tiles.__setitem__(
                        ("q", t), qk_sq("q", t, True)))
                    post(base + 3 * j + 2, lambda t=t16: qk_red(
                        "q", t, q2_tiles.pop(("q", t))))
                post(base + 12, lambda g=g4: tr_group("q", g, evac_act=True))

            if ab is None:
                pre(40, load_wpm)

            # Pipeline: S@i, exp@i-1, PV@i-2 -- by the time PV(i-2) and
            # S(i) reach the PE FIFO their sems have already fired, so the
            # PE never stalls mid-stream and the exp stream runs at the Act
            # engine's native rate.
            proj_pending = []
            pend_exp = None          # step awaiting exp
            pend_pv = None           # (pg, step) awaiting PV
            pend_div = []            # divide stage-B args, run next iteration

            def do_pv(pg, st):
                emit_pv(pg, *st)
                if st[2:] == (7, 1) and ab != "pipe":
                    php, pqb = st[:2]
                    pend_div.append(emit_divide_a(php, pqb))
                    if php == NHP - 1 and pqb < 3 and ab in (None, "proj"):
                        for m in range(8):
                            proj_pending.append(
                                lambda m=m, n=pqb: proj_group(m, n))

            for i, st in enumerate(steps):
                hp, qb_i, g, h = st
                if (g, h) == (0, 0):
                    pvs_all[hp, qb_i] = [
                        ps_pv.tile([128, 512], F32, tag="pv",
                                   name=f"r{rep}pv{hp}{qb_i}{_h}")
                        for _h in range(2)]
                emit_s(*st)
                if pend_div:
                    emit_divide_b(*pend_div.pop(0))
                for t in fill_pre.get(i, ()):
                    t()
                if pend_pv is not None:
                    do_pv(*pend_pv)
                    pend_pv = None
                if pend_exp is not None:
                    pend_pv = (emit_exp(*pend_exp), pend_exp)
                pend_exp = st
                for t in fill_post.get(i, ()):
                    t()
                if proj_pending and i % 2 == 0:
                    proj_pending.pop(0)()
            # drain
            last_pg = emit_exp(*pend_exp)
            do_pv(*pend_pv)
            do_pv(last_pg, pend_exp)
            for d in pend_div:
                emit_divide_b(*d)
            for t in proj_pending:
                t()
            if ab in (None, "proj"):
                for m in range(8):
                    proj_group(m, 3)
            elif ab == "pipe":
                for h in range(2):
                    sod = scr_p.tile([128, 512], F32, tag="so", bufs=4,
                                     name=f"r{rep}abl{h}")
                    nc.vector.tensor_copy(sod[:], pvs_all[3, 3][h][:])
                    nc.sync.dma_start(outT[128 * h:128 * (h + 1), 0:512],
                                      sod[:])
            elif ab == "div":
                if _DIVMODE == "full":
                    for hp in range(NHP):
                        sod = scr_p.tile([128, 512], F32, tag="so", bufs=4,
                                         name=f"r{rep}abl{hp}")
                        nc.vector.tensor_copy(sod[:], oTs[hp][:, 0:512])
                        nc.sync.dma_start(
                            outT[128 * hp:128 * (hp + 1), 0:512], sod[:])
                else:
                    nc.sync.dma_start(outT[0:1, 0:16], sink[:])

    nc.compile()
    return nc


def make_in_maps(x, w_qkv, b_qkv, g_q, g_k, w_proj, b_proj):
    """Host-side sharding: per-core input dict (weights pre-cast to bf16)."""
    import ml_dtypes
    f32 = np.float32
    bf16 = ml_dtypes.bfloat16
    x = np.ascontiguousarray(x, dtype=f32)
    w_qkv = np.asarray(w_qkv, dtype=f32)
    b_qkv = np.asarray(b_qkv, dtype=f32)
    g_q = np.asarray(g_q, dtype=f32)
    g_k = np.asarray(g_k, dtype=f32)
    w_proj = np.asarray(w_proj, dtype=f32)
    b_proj = np.asarray(b_proj, dtype=f32)

    ident = np.eye(128, dtype=f32)
    onescol = np.ones((1, 128), f32)
    ones1 = np.ones((1, 128), f32)
    # g/64 per d-row of a head pair (folds the tdif = 64*(q-mu) scaling)
    gqc = np.concatenate([g_q, g_q]).reshape(128, 1) * (1.0 / 64.0)
    gkc = np.concatenate([g_k, g_k]).reshape(128, 1) * (1.0 / 64.0)

    in_maps = []
    for c in range(NCORES):
        b = c // 2
        hg = c % 2
        cs = slice(hg * 512, (hg + 1) * 512)
        bv = b_qkv[2 * C:][cs]
        wp_half = w_proj[hg * 512:(hg + 1) * 512, :]
        bp_eff = bv @ wp_half + (b_proj if hg == 0 else 0.0)
        bqk_rows = np.stack([b_qkv[cs], b_qkv[C:][cs]])
        in_maps.append({
            "xT": np.ascontiguousarray(x[b].T).astype(bf16),
            "wq": np.ascontiguousarray(w_qkv[:, cs]).astype(bf16),
            "wk": np.ascontiguousarray(w_qkv[:, C:][:, cs]).astype(bf16),
            "wv": np.ascontiguousarray(w_qkv[:, 2 * C:][:, cs]).astype(bf16),
            "wp": np.ascontiguousarray(wp_half).astype(bf16),
            "bqk": bqk_rows.astype(bf16),
            "onescol": onescol.astype(bf16),
            "ident": ident.astype(bf16),
            "gqc": gqc,
            "gkc": gkc,
            "ones1": ones1,
            "epsc": np.full((128, 1), EPS, f32),
            "bp": np.ascontiguousarray(bp_eff.reshape(8, 128).T.astype(f32)),
        })
    return in_maps


def unshard(partials):
    """partials: list of 8 outT arrays [C, SEQ] -> full [B, SEQ, C]."""
    out = np.empty((B, SEQ, C), np.float32)
    for b in range(B):
        out[b] = (partials[2 * b] + partials[2 * b + 1]).T
    return out


def kernel(x, w_qkv, b_qkv, g_q, g_k, w_proj, b_proj):
    from concourse.bass_utils import run_bass_kernel_spmd

    # q/k biases feed the pre-LN values; emit the bias-prefill matmuls only
    # if they are actually nonzero (the spec fills b_qkv with zeros)
    wb = bool(np.any(np.asarray(b_qkv)[:2 * C]))
    key = ("nc", wb)
    if key not in _CACHE:
        _CACHE[key] = _build_nc(with_bias=wb)
    nc = _CACHE[key]
    in_maps = make_in_maps(x, w_qkv, b_qkv, g_q, g_k, w_proj, b_proj)
    res = run_bass_kernel_spmd(nc, in_maps, list(range(NCORES)))
    return unshard([res.results[c]["outT"] for c in range(NCORES)])
